# revision 1
# baseline (speedup 1.0000x reference)
"""Chunked attention Trainium2 Bass kernel.

Problem: B=2, S=8192, HIDDEN=1024, HEADS=16, HEAD_DIM=64, CHUNK=2048,
OVERLAP=128. Sharding: head-parallel x batch-parallel -> 32 (b,h) jobs,
4 per core on 8 cores. Each core computes full-seq chunked attention for
its 4 heads; the host slices/pre-transposes inputs and reassembles the
output.

Per-core dataflow (fp32 / float32r):
  - Host supplies Q^T and K^T in [d, seq] layout, duplicated across both
    64-partition halves (rows 0:64 == rows 64:128) so K_c=64 matmuls can
    be row-packed in pairs (two concurrent matmuls in the PE array).
  - QK^T: S^T[k,q] float32r matmuls into PSUM groups of 3 banks.
  - One ACT exp per group (scale=1/8 folded into the activation) ->
    P^T in SBUF (float32r; ACT does not actually round the values).
  - PV: lhsT=[V|1] (65 cols, stationary) accumulates [O^T; l] into one
    PSUM bank over all k-tiles of the chunk.
  - Device returns the UNNORMALIZED per-chunk [O^T; l] (65 rows per
    chunk, concatenated along seq); softmax division and the 128-wide
    overlap-band blending happen on the host in fp32.
"""

import sys

if '/opt/trn_rl_repo' not in sys.path:
    sys.path.insert(0, '/opt/trn_rl_repo')

import numpy as np

import concourse.bass as bass
import concourse.mybir as mybir
import concourse.tile as tile
from concourse.bass_utils import run_bass_kernel_spmd

F32 = mybir.dt.float32
F32R = mybir.dt.float32r
EXP = mybir.ActivationFunctionType.Exp

B, S, HIDDEN, HEADS, HD = 2, 8192, 1024, 16, 64
SCALE = 1.0 / 8.0  # 1/sqrt(64)
N_CORES = 8
JOBS = 4  # (b, h) pairs per core
# (q0, Lq, k0, Lk) per chunk; step=1920, overlap=128
CHUNKS = [
    (0, 2048, 0, 2176),
    (1920, 2048, 1792, 2304),
    (3840, 2048, 3712, 2304),
    (5760, 2048, 5632, 2304),
    (7680, 512, 7552, 640),
]
COLS = [0, 2048, 4096, 6144, 8192]  # chunk col offsets in the out buffer
SQ = 8704  # sum of chunk Lq
GROUP = 3  # k-tiles per S^T PSUM group (3 banks x2 bufs + opsum x2 = 8)


def _legalize_waits(nc, max_waits=1):
    """walrus in this config rejects >1 sync-wait per instruction: hoist
    excess waits onto injected same-engine NoOps placed just before."""
    cnt = 0
    for f in nc.m.functions:
        for blk in f.blocks:
            il = blk.instructions
            if not any(
                i.sync_info is not None and i.sync_info.on_wait
                and len(i.sync_info.on_wait) > max_waits for i in il
            ):
                continue
            new = []
            for inst in il:
                si = inst.sync_info
                if si is not None and si.on_wait and len(si.on_wait) > max_waits:
                    waits = list(si.on_wait)
                    spill, keep = waits[:-max_waits], waits[-max_waits:]
                    for w in spill:
                        nop = mybir.InstNoOp(
                            name=f"I-wsplit-{cnt}", ins=[], outs=[])
                        cnt += 1
                        nop.engine = inst.engine
                        nop.sync_info = mybir.SyncInfo(on_wait=[w], on_update=[])
                        new.append(nop)
                    inst.sync_info = mybir.SyncInfo(
                        on_wait=keep, on_update=list(si.on_update or []))
                new.append(inst)
            blk.instructions = new
    return cnt


def _build_nc(reps=1):
    nc = bass.Bass()
    qt_in = nc.declare_dram_parameter("qt", [JOBS, 128, S], F32, isOutput=False)
    kt_in = nc.declare_dram_parameter("kt", [JOBS, 128, S], F32, isOutput=False)
    v_in = nc.declare_dram_parameter("v", [JOBS, S, HD], F32, isOutput=False)
    out = nc.declare_dram_parameter("out", [JOBS, 65, SQ], F32, isOutput=True)

    with tile.TileContext(nc) as tc:
        with (
            tc.tile_pool(name="const", bufs=1) as cpool,
            tc.tile_pool(name="ops", bufs=2) as ops,          # qT/kT/vW
            tc.tile_pool(name="probs", bufs=7) as probs,      # pT
            tc.tile_pool(name="opath", bufs=3) as opath,      # o_sb staging
            tc.tile_pool(name="spsum", bufs=2, space="PSUM") as spsum,
            tc.tile_pool(name="onepsum", bufs=2, space="PSUM") as onepsum,
        ):
            ones_f32 = cpool.tile([128, 1], F32)
            nc.vector.memset(ones_f32, 1.0)

            for j in [jj for _ in range(reps) for jj in range(JOBS)]:
                for ci, (q0, lq, k0, lk) in enumerate(CHUNKS):
                    nk = lk // 128

                    qT = ops.tile([128, lq], F32R, tag="qT")
                    nc.sync.dma_start(
                        out=qT, in_=qt_in[j, :, q0:q0 + lq].bitcast(F32R))
                    kT = ops.tile([128, lk], F32R, tag="kT")
                    nc.sync.dma_start(
                        out=kT, in_=kt_in[j, :, k0:k0 + lk].bitcast(F32R))
                    vW = ops.tile([128, nk * 65], F32R, tag="vW")
                    vv = vW.rearrange("p (t e) -> p t e", e=65)
                    nc.sync.dma_start(
                        out=vv[:, :, 0:64],
                        in_=v_in[j, k0:k0 + lk, :].rearrange(
                            "(t p) d -> p t d", p=128).bitcast(F32R),
                    )
                    nc.vector.tensor_copy(
                        vv[:, :, 64], ones_f32.broadcast_to([128, nk]))

                    ngroups = (nk + GROUP - 1) // GROUP
                    for qb in range(lq // 512):
                        qs = slice(qb * 512, qb * 512 + 512)
                        opsum = onepsum.tile([128, 512], F32, tag="opsum")
                        # emit all QK+exp for the q-block first so the PE
                        # always feeds ACT before doing PV work (PV fills
                        # PE gaps at lower scheduler priority)
                        pTs = []
                        for g in range(ngroups):
                            kts = list(range(g * GROUP, min((g + 1) * GROUP, nk)))
                            sp = spsum.tile([128, 512 * GROUP], F32, tag="sp")
                            # QK^T: S^T[k,q]; consecutive k-tiles alternate
                            # row halves -> pairs run concurrently in PE
                            for i, kt in enumerate(kts):
                                rows = slice(64 * (kt % 2), 64 * (kt % 2) + 64)
                                nc.tensor.matmul(
                                    sp[:, i * 512:(i + 1) * 512],
                                    kT[rows, kt * 128:(kt + 1) * 128],
                                    qT[rows, qs],
                                    start=True, stop=True,
                                    tile_position=(64 * (kt % 2), 0),
                                    skip_group_check=True,
                                )
                            pT = probs.tile([128, 512 * GROUP], F32R, tag="pT")
                            nw = 512 * len(kts)
                            nc.scalar.activation(
                                pT[:, 0:nw], sp[:, 0:nw], EXP, scale=SCALE)
                            pTs.append((kts, pT))
                        for kts, pT in pTs:
                            for i, kt in enumerate(kts):
                                nc.tensor.matmul(
                                    opsum[0:65, :],
                                    vW[:, kt * 65:(kt + 1) * 65],
                                    pT[:, i * 512:(i + 1) * 512],
                                    start=(kt == 0), stop=(kt == nk - 1),
                                    skip_group_check=True,
                                )
                        o_sb = opath.tile([65, 512], F32, tag="osb")
                        nc.vector.tensor_copy(o_sb, opsum[0:65, :])
                        c0 = COLS[ci] + qb * 512
                        nc.sync.dma_start(
                            out=out[j, :, c0:c0 + 512], in_=o_sb)

    _legalize_waits(nc)
    return nc


_NC = None


def _get_nc():
    global _NC
    if _NC is None:
        _NC = _build_nc()
    return _NC


def make_in_maps(query, key_, value):
    """Host-side prep: per-core slices; Q^T/K^T in [d, seq] layout
    duplicated across both partition halves."""
    qh = query.reshape(B, S, HEADS, HD)
    kh = key_.reshape(B, S, HEADS, HD)
    vh = value.reshape(B, S, HEADS, HD)
    qT = np.ascontiguousarray(qh.transpose(0, 2, 3, 1))  # [B, H, D, S]
    kT = np.ascontiguousarray(kh.transpose(0, 2, 3, 1))
    in_maps = []
    for c in range(N_CORES):
        jobs = [(g // HEADS, g % HEADS) for g in range(4 * c, 4 * c + 4)]
        qt_c = np.empty((JOBS, 128, S), np.float32)
        kt_c = np.empty((JOBS, 128, S), np.float32)
        v_c = np.empty((JOBS, S, HD), np.float32)
        for jj, (b, h) in enumerate(jobs):
            qt_c[jj, 0:64] = qT[b, h]
            qt_c[jj, 64:128] = qT[b, h]
            kt_c[jj, 0:64] = kT[b, h]
            kt_c[jj, 64:128] = kT[b, h]
            v_c[jj] = vh[b, :, h]
        in_maps.append({"qt": qt_c, "kt": kt_c, "v": v_c})
    return in_maps


def assemble_out(results):
    """Host: per-chunk softmax division + overlap-band blending (fp32,
    mirrors the reference's merge), then scatter into [B, S, HIDDEN]."""
    wt = np.linspace(1.0, 0.0, 128).astype(np.float32)  # prev-chunk tail
    wh = np.linspace(0.0, 1.0, 128).astype(np.float32)  # cur-chunk head
    denom = (wt + wh) + np.float32(1e-10)
    a = (wt / denom).astype(np.float32)[:, None]
    bb = (wh / denom).astype(np.float32)[:, None]

    out = np.empty((B, S, HIDDEN), dtype=np.float32)
    for c in range(N_CORES):
        oc = results[c]["out"]  # [4, 65, SQ]
        for jj, g in enumerate(range(4 * c, 4 * c + 4)):
            b, h = g // HEADS, g % HEADS
            full = np.empty((S, HD), np.float32)
            prev_tail = None
            for ci, (q0, lq, k0, lk) in enumerate(CHUNKS):
                off = COLS[ci]
                blk = oc[jj, :, off:off + lq]
                on = (blk[0:64] / blk[64:65]).T  # [lq, 64] normalized
                lo = 0
                if ci > 0:
                    full[q0:q0 + 128] = prev_tail * a + on[0:128] * bb
                    lo = 128
                hi = lq
                if ci < len(CHUNKS) - 1:
                    hi = lq - 128
                    prev_tail = on[lq - 128:lq]
                full[q0 + lo:q0 + hi] = on[lo:hi]
            out[b, :, h * HD:(h + 1) * HD] = full
    return out


def kernel(query, key, value):
    query = np.asarray(query, dtype=np.float32)
    key_ = np.asarray(key, dtype=np.float32)
    value = np.asarray(value, dtype=np.float32)
    nc = _get_nc()
    in_maps = make_in_maps(query, key_, value)
    res = run_bass_kernel_spmd(nc, in_maps, list(range(N_CORES)))
    return assemble_out(res.results)

